# revision 4
# baseline (speedup 1.0000x reference)
"""LSTM cell kernel for Trainium2, SPMD over 8 NeuronCores.

Problem: nn_LstmCell — B=8192, D_IN=D_H=2048.
    g = x @ Wx.T + bx + h @ Wh.T + bh          # [B, 3H]
    gi, gm, go = split(g, 3)
    c_new = sigmoid(gm)*c + sigmoid(gi)*tanh(gm)
    h_new = sigmoid(go)*tanh(c_new)

Strategy:
  - Data-parallel over batch: each core owns 1024 rows of x/h/c.
  - Fused GEMM computed transposed (gates on PSUM partitions, batch on the
    free dim) so per-gate biases fold into the ScalarE activation.
  - Mixed precision split by gate sensitivity: the m-gate pre-activation
    feeds tanh (derivative ~1) so it runs in bf16; the i/o gates only feed
    sigmoid (derivative <= 0.25) so they run in fp8 e4m3 with DoubleRow
    perf mode (K=256 per matmul, 2x PE throughput). Measured end-to-end
    rel err ~1.4e-2 vs 2.6e-2 for all-fp8 (gate: 2e-2).
  - fp8 scaling: A*32, W*512 (W's absmax 0.022 is below e4m3's min normal
    0.0156, so unscaled W would quantize to subnormals). The 1/16384
    descale folds into the sigmoid activation's scale operand.
  - DoubleRow matmuls may only write PSUM partitions 0..63, so each i/o
    gate accumulates its two 64-row halves into separate [64, 512] banks;
    the sigmoid ACTs then write the two partition halves of one [128, 512]
    SBUF tile (ScalarE supports input/output partition-base offsets).
  - Weights streamed from HBM (one pass), activations resident in SBUF.

Host-side: layout transforms + bf16/fp8 casts (not counted in HW exec time).
"""

import os

import numpy as np
import ml_dtypes

N_CORES = 8
B = 8192
DH = 2048            # latent dim (= D_IN = D_H)
K = 2 * DH           # 4096 contraction dim
BLOC = B // N_CORES  # 1024 batch rows per core
P = 128
KT = K // P          # 32 k-tiles (128 each)
KG = KT // 2         # 16 doubled k-groups (256 each) for fp8 DoubleRow
DTL = DH // P        # 16 d-tiles per gate
NF = 512             # matmul free dim (one PSUM bank of fp32)
NH = BLOC // NF      # 2 batch halves (vtiles per d-tile)

N_M8 = 5             # leading m-gate d-tiles computed in fp8 (see below)
SA = 32.0            # fp8 activation scale
SW = 512.0           # fp8 weight scale
IO_DESCALE = 1.0 / (SA * SW)

_F8 = ml_dtypes.float8_e4m3

_CACHE = {}
LAST_RESULT = None  # BassKernelResults from the most recent run (for test.py)


def _split_multiwaits(nc):
    """This container's walrus build rejects >1 sync-wait on an engine
    instruction ("Too many sync wait commands"). Split extra waits into
    standalone EventSemaphore instructions on the same engine immediately
    before the instruction (same stall semantics: engines are in-order)."""
    import concourse.mybir as mybir

    f = nc.m.functions[0]
    for blk in f.blocks:
        new_insts = []
        for inst in blk.instructions:
            si = getattr(inst, "sync_info", None)
            ow = list(si.on_wait) if (si is not None and si.on_wait) else []
            if len(ow) > 1:
                for w in ow[:-1]:
                    new_insts.append(
                        mybir.InstEventSemaphore(
                            name=nc.get_next_instruction_name(),
                            engine=inst.engine,
                            ins=[],
                            outs=[],
                            sync_info=mybir.SyncInfo(on_wait=[w], on_update=[]),
                        )
                    )
                inst.sync_info = mybir.SyncInfo(
                    on_wait=[ow[-1]], on_update=list(si.on_update)
                )
            new_insts.append(inst)
        blk.instructions[:] = new_insts


def _build_bass(dtl=DTL):
    import concourse.bass as bass
    import concourse.mybir as mybir
    import concourse.tile as tile

    f32 = mybir.dt.float32
    f16 = mybir.dt.float16
    f8 = mybir.dt.float8e4
    AF = mybir.ActivationFunctionType
    DR = mybir.MatmulPerfMode.DoubleRow

    nc = bass.Bass("TRN2", name="lstm_cell")

    # m-gate weights, fp16: WM[d, p, kt, m] = Wm[d*128+m, kt*128+p]
    WM = nc.dram_tensor("WM", [dtl, P, KT, P], f16, kind="ExternalInput")
    # i/o gate weights, fp8 DoubleRow layout, plus N_M8 extra strips
    # (index 2*dtl+d) holding the M-GATE's first N_M8 d-tiles in fp8:
    # those tiles run all-fp8, halving their m-matmul time and decoupling
    # the prologue from the 8 MB bf16 A stream (which then loads during
    # the fp8 tiles instead of stalling the pipeline). Each converted tile
    # costs ~1e-3 of rel err (measured: 1 -> 1.51e-2, 3 -> 1.69e-2).
    # W8[g*16+d, p, kg, ii, m] = Wg[d*128+m, kg*256+ii*128+p] * SW
    W8 = nc.dram_tensor("W8", [2 * dtl + N_M8, P, KG, 2, P], f8, kind="ExternalInput")
    # activations A = [x ‖ h], twice: fp16 for m-gate, fp8*SA for i/o
    ABF = nc.dram_tensor("ABF", [P, KT, BLOC], f16, kind="ExternalInput")
    A8 = nc.dram_tensor("A8", [P, KT, BLOC], f8, kind="ExternalInput")
    CT = nc.dram_tensor("CT", [DH, BLOC], f32, kind="ExternalInput")
    BIAS = nc.dram_tensor("BIAS", [P, 3 * dtl], f32, kind="ExternalInput")
    # i/o biases regrouped per 64-row half (DoubleRow outputs sit at
    # partitions 0..63): BIO[p, ((g*16+d)*2)+b] = bias_g[d*128+b*64+p].
    # Cols 4*dtl + 2*d + b hold the m-gate's bias halves for the N_M8
    # fp8 m-tiles.
    BIO = nc.dram_tensor("BIO", [64, 4 * dtl + 2 * N_M8], f32, kind="ExternalInput")
    HT = nc.dram_tensor("HT", [DH, BLOC], f32, kind="ExternalOutput")
    CNT = nc.dram_tensor("CNT", [DH, BLOC], f32, kind="ExternalOutput")

    with tile.TileContext(nc) as tc:
        with (
            tc.tile_pool(name="const", bufs=1) as const_pool,
            tc.tile_pool(name="wpool", bufs=2) as wpool,
            tc.tile_pool(name="cpool", bufs=2) as cpool,
            tc.tile_pool(name="epool", bufs=3) as epool,
            tc.tile_pool(name="psum", bufs=1, space="PSUM") as psum_pool,
        ):
            # Activations resident in SBUF; per-k-chunk loads so the first
            # d-tile's matmuls can start as soon as early chunks land.
            # Both A streams go on the gpsimd queue, fp8 strictly first:
            # the model serializes all transfers through one shared DMA
            # resource, so only queue order keeps the big bf16 chunks (m-gate,
            # consumed last) from starving the fp8 chunks (i/o, consumed
            # first). Chunks sized to amortize the ~1us SWDGE prep per DMA.
            a8_sb = const_pool.tile([P, KT, BLOC], f8, name="a8_sb")
            for ch in range(8):
                nc.gpsimd.dma_start(
                    a8_sb[:, 4 * ch : 4 * ch + 4, :],
                    A8[:, 4 * ch : 4 * ch + 4, :],
                )
            # ABF chunk DMAs are emitted inside the d==0 body, after d0's
            # wm/c loads, so those beat the bulk bf16 stream to the DMA
            # engines (the m-gate is scheduled last within d0 anyway).
            abf_sb = const_pool.tile([P, KT, BLOC], f16, name="abf_sb")
            # bias loads are emitted inside the d==0 body, after the first
            # weight strips: they aren't needed until the first epilogue,
            # and ahead of the strips they'd burn ~1.3us of HWDGE setup on
            # the critical path to the first matmul.
            bias_sb = const_pool.tile([P, 3 * dtl], f32, name="bias_sb")
            bio_sb = const_pool.tile([64, 4 * dtl + 2 * N_M8], f32, name="bio_sb")

            for d in range(dtl):
                # Stream this d-tile's weight strips: fp8 i/o (0.5 MB each)
                # + bf16 m (1 MB).
                # d0's c rides the gpsimd queue between the A8 and ABF
                # streams: it would otherwise win the shared DMA engines
                # ahead of the latency-critical fp8 chunks.
                d0_eng = nc.gpsimd if d == 0 else nc.sync
                m_fp8 = d < min(N_M8, dtl)
                gates = [("i", d), ("o", dtl + d)]
                if m_fp8:
                    # i, m, o order matches d0's kg-major gate order
                    gates.insert(1 if d == 0 else 2, ("m", 2 * dtl + d))
                w8 = {}
                for g, idx in gates:
                    w8[g] = wpool.tile([P, KG, 2, P], f8, name=f"w8{g}", tag=f"w8{g}")
                    if d == 0:
                        # halves so the first kg's weights land sooner
                        for hf in range(2):
                            nc.sync.dma_start(
                                w8[g][:, 8 * hf : 8 * hf + 8],
                                W8[idx][:, 8 * hf : 8 * hf + 8],
                            )
                    else:
                        nc.sync.dma_start(w8[g][:], W8[idx])
                if not m_fp8:
                    wm = wpool.tile([P, KT, P], f16, name="wm", tag="wm")
                    # First bf16 tile's strip via gpsimd (post-ABF, idle Pool
                    # queue): on sync it sits behind the previous tile's
                    # output DMAs and arrives ~1.4us late.
                    weng = nc.gpsimd if d == min(N_M8, DTL) else nc.sync
                    weng.dma_start(wm[:], WM[d])

                c_tiles, psums = {}, {}
                for nh in range(NH):
                    c_t = cpool.tile([P, NF], f32, name=f"c_{nh}", tag=f"c_{nh}")
                    d0_eng.dma_start(
                        c_t[:], CT[d * P : (d + 1) * P, nh * NF : (nh + 1) * NF]
                    )
                    c_tiles[nh] = c_t
                    # m-gate: one full bank; parity tags so consecutive
                    # vtiles overlap. i/o: [64, 512] banks (DoubleRow dst
                    # partition must be 0), one per 64-row half. d0's fp8
                    # m-gate gets its own [64, 512] pair (8 banks total).
                    if m_fp8:
                        for b in range(2):
                            psums[("m8", nh, b)] = psum_pool.tile(
                                [64, NF], f32, name=f"ps_m8{b}", tag=f"ps_m8{b}"
                            )
                    else:
                        psums[("m", nh)] = psum_pool.tile(
                            [P, NF], f32, name=f"ps_m{nh}", tag=f"ps_m{nh}"
                        )
                    for g in "io":
                        if d == 0 and g == "i" and nh == 1:
                            continue  # pm-backed, allocated in the d0 branch
                        for b in range(2):
                            psums[(g, nh, b)] = psum_pool.tile(
                                [64, NF], f32, name=f"ps_{g}{b}", tag=f"ps_{g}{b}"
                            )

                def io_matmul(g, nh, b, kg, key=None):
                    # fp8 DoubleRow: K=256 (k-tile pair), M=64, N=512.
                    nc.tensor.matmul(
                        psums[(key or g, nh, b)],
                        w8[g][:, kg, :, b * 64 : (b + 1) * 64],
                        a8_sb[:, 2 * kg : 2 * kg + 2, nh * NF : (nh + 1) * NF],
                        start=(kg == 0),
                        stop=(kg == KG - 1),
                        perf_mode=DR,
                    )

                def m_matmul(nh, kt):
                    nc.tensor.matmul(
                        psums[("m", nh)][:],
                        wm[:, kt, :],
                        abf_sb[:, kt, nh * NF : (nh + 1) * NF],
                        start=(kt == 0),
                        stop=(kt == KT - 1),
                    )

                def sig_io(g, gi, s_g, nh, b):
                    nc.scalar.activation(
                        s_g[b * 64 : (b + 1) * 64, :],
                        psums[(g, nh, b)],
                        AF.Sigmoid,
                        bias=bio_sb[
                            :, (gi * dtl + d) * 2 + b : (gi * dtl + d) * 2 + b + 1
                        ],
                        scale=IO_DESCALE,
                    )

                def epilogue(nh):
                    # Emission order matters: engines are in-order, so the
                    # o-dependent ops (s_o, h_new) go last — everything else
                    # completes during the o-gate matmuls and only the short
                    # s_o -> h_new chain trails the final matmul.
                    b_m = bias_sb[:, dtl + d : dtl + d + 1]

                    s_i = epool.tile([P, NF], f32, name="s_i", tag="s_i")
                    t_m = epool.tile([P, NF], f32, name="t_m", tag="t_m")
                    s_m = epool.tile([P, NF], f32, name="s_m", tag="s_m")
                    s_o = epool.tile([P, NF], f32, name="s_o", tag="s_o")
                    part = epool.tile([P, NF], f32, name="part", tag="part")
                    fc = epool.tile([P, NF], f32, name="fc", tag="fc")
                    c_new = epool.tile([P, NF], f32, name="c_new", tag="c_new")
                    t_c = epool.tile([P, NF], f32, name="t_c", tag="t_c")
                    h_new = epool.tile([P, NF], f32, name="h_new", tag="h_new")

                    # i halves: PSUM [64, 512] at partition base 0 ->
                    # partition halves of the [128, 512] SBUF tile.
                    for b in range(2):
                        sig_io("i", 0, s_i, nh, b)
                    if m_fp8:
                        for b in range(2):
                            col = 4 * dtl + 2 * d + b
                            bm8 = bio_sb[:, col : col + 1]
                            nc.scalar.activation(
                                t_m[b * 64 : (b + 1) * 64, :],
                                psums[("m8", nh, b)][:],
                                AF.Tanh, bias=bm8, scale=IO_DESCALE,
                            )
                            nc.scalar.activation(
                                s_m[b * 64 : (b + 1) * 64, :],
                                psums[("m8", nh, b)][:],
                                AF.Sigmoid, bias=bm8, scale=IO_DESCALE,
                            )
                    else:
                        nc.scalar.activation(t_m[:], psums[("m", nh)][:], AF.Tanh, bias=b_m)
                        nc.scalar.activation(s_m[:], psums[("m", nh)][:], AF.Sigmoid, bias=b_m)
                    nc.vector.tensor_mul(part[:], s_i[:], t_m[:])
                    nc.vector.tensor_mul(fc[:], s_m[:], c_tiles[nh][:])
                    nc.vector.tensor_add(c_new[:], fc[:], part[:])
                    nc.scalar.activation(t_c[:], c_new[:], AF.Tanh)
                    nc.sync.dma_start(
                        CNT[d * P : (d + 1) * P, nh * NF : (nh + 1) * NF], c_new[:]
                    )
                    if d == dtl - 1 and nh == NH - 1:
                        # Final vtile: halve the o-dependent chain so the
                        # first h_new DMA overlaps the second half's compute.
                        for hf in range(2):
                            cs = hf * (NF // 2)
                            for b in range(2):
                                col = (dtl + d) * 2 + b
                                nc.scalar.activation(
                                    s_o[b * 64 : (b + 1) * 64, cs : cs + NF // 2],
                                    psums[("o", nh, b)][:, cs : cs + NF // 2],
                                    AF.Sigmoid,
                                    bias=bio_sb[:, col : col + 1],
                                    scale=IO_DESCALE,
                                )
                            nc.vector.tensor_mul(
                                h_new[:, cs : cs + NF // 2],
                                s_o[:, cs : cs + NF // 2],
                                t_c[:, cs : cs + NF // 2],
                            )
                            heng = nc.scalar if hf == 0 else nc.sync
                            heng.dma_start(
                                HT[d * P : (d + 1) * P,
                                   nh * NF + cs : nh * NF + cs + NF // 2],
                                h_new[:, cs : cs + NF // 2],
                            )
                    else:
                        for b in range(2):
                            sig_io("o", 1, s_o, nh, b)
                        nc.vector.tensor_mul(h_new[:], s_o[:], t_c[:])
                        nc.sync.dma_start(
                            HT[d * P : (d + 1) * P, nh * NF : (nh + 1) * NF], h_new[:]
                        )

                if d == 0:
                    nc.sync.dma_start(bias_sb[:], BIAS[:])
                    nc.sync.dma_start(bio_sb[:], BIO[:])
                    # Emit the bulk bf16 A stream (needed first by the first
                    # bf16 m-gate) behind d0's c loads on the same queue.
                    for ch in range(8):
                        nc.gpsimd.dma_start(
                            abf_sb[:, 4 * ch : 4 * ch + 4, :],
                            ABF[:, 4 * ch : 4 * ch + 4, :],
                        )
                    # d0 is all-fp8 (m included): kg-major ACROSS gates so
                    # every fp8 A chunk feeds matmuls on arrival and the PE
                    # tracks the DMA stream without long stalls; no
                    # dependence on the bf16 stream at all. vtile 1's i-gate
                    # rides along in the otherwise-idle pm banks ([64, 512]
                    # at base 0) to soak up the DMA-pacing idle.
                    for b in range(2):
                        t = psum_pool.tile([P, NF], f32, name=f"ps_m{b}", tag=f"ps_m{b}")
                        psums[("i", 1, b)] = t[0:64, :]
                    for kg in range(KG):
                        for g in "imo":
                            for b in range(2):
                                io_matmul(g, 0, b, kg, key="m8" if g == "m" else None)
                        for b in range(2):
                            io_matmul("i", 1, b, kg)
                    epilogue(0)
                    for kg in range(KG):
                        for g in "mo":
                            for b in range(2):
                                io_matmul(g, 1, b, kg, key="m8" if g == "m" else None)
                    epilogue(1)
                else:
                    # gate-major per vtile, o last: everything except the
                    # short s_o -> h_new chain completes during the o-gate
                    # matmuls (see epilogue()).
                    for nh in range(NH):
                        for b in range(2):
                            for kg in range(KG):
                                io_matmul("i", nh, b, kg)
                        if m_fp8:
                            for b in range(2):
                                for kg in range(KG):
                                    io_matmul("m", nh, b, kg, key="m8")
                        else:
                            for kt in range(KT):
                                m_matmul(nh, kt)
                        for b in range(2):
                            for kg in range(KG):
                                io_matmul("o", nh, b, kg)
                        epilogue(nh)

    _split_multiwaits(nc)
    return nc


def _get_bass():
    if "nc" not in _CACHE:
        _CACHE["nc"] = _build_bass()
    return _CACHE["nc"]


def _prepare_in_maps(x, h, c, Wix, bix, Wmx, bmx, Wox, box, Wih, bih, Wmh, bmh, Woh, boh):
    x = np.asarray(x, dtype=np.float32)
    h = np.asarray(h, dtype=np.float32)
    c = np.asarray(c, dtype=np.float32)

    # Per-gate fused weights [2048, 4096]: W = [Wx ‖ Wh]
    Wg = {
        "i": np.concatenate([np.asarray(Wix), np.asarray(Wih)], axis=1),
        "m": np.concatenate([np.asarray(Wmx), np.asarray(Wmh)], axis=1),
        "o": np.concatenate([np.asarray(Wox), np.asarray(Woh)], axis=1),
    }

    # m-gate bf16: WM[d, p, kt, m] = Wm[d*128+m, kt*128+p]
    WM_host = np.ascontiguousarray(
        Wg["m"].astype(np.float32).reshape(DTL, P, KT, P).transpose(0, 3, 2, 1)
    ).astype(np.float16)

    # i/o gates fp8 (scaled by SW), DoubleRow layout:
    # W8[g*16+d, p, kg, ii, m] = Wg[d*128+m, kg*256+ii*128+p]*SW
    # plus the m-gate's d=0 strip at index 2*DTL (d-tile 0 runs all-fp8).
    w8_list = []
    for g in "io":
        ws = (Wg[g].astype(np.float32) * SW).astype(_F8)
        w8_list.append(ws.reshape(DTL, P, KG, 2, P).transpose(0, 4, 2, 3, 1))
    wm8 = (Wg["m"][: N_M8 * P].astype(np.float32) * SW).astype(_F8)
    w8_list.append(wm8.reshape(N_M8, P, KG, 2, P).transpose(0, 4, 2, 3, 1))
    W8_host = np.ascontiguousarray(np.concatenate(w8_list, axis=0))

    # A = [x ‖ h] : [8192, 4096] -> per-core [p, kt, n], in bf16 and fp8*SA
    A = np.concatenate([x, h], axis=1)
    A_t = A.reshape(N_CORES, BLOC, KT, P).transpose(0, 3, 2, 1)
    ABF_host = np.ascontiguousarray(A_t).astype(np.float16)
    A8_host = np.ascontiguousarray(A_t * np.float32(SA)).astype(_F8)

    # c transposed per core: [core, 2048, 1024]
    CT_host = np.ascontiguousarray(c.reshape(N_CORES, BLOC, DH).transpose(0, 2, 1))

    bias = {g: (np.asarray(bx) + np.asarray(bh)).astype(np.float32)
            for g, bx, bh in (("i", bix, bih), ("m", bmx, bmh), ("o", box, boh))}
    BIAS_host = np.ascontiguousarray(
        np.concatenate([bias["i"], bias["m"], bias["o"]]).reshape(3 * DTL, P).T
    )
    # BIO[p, (g*16+d)*2+b] = bias_g[d*128+b*64+p] for g in (i, o);
    # trailing 2*N_M8 cols: m-gate bias halves for the fp8 m-tiles.
    BIO_host = np.ascontiguousarray(
        np.concatenate([bias["i"], bias["o"], bias["m"][: N_M8 * P]])
        .reshape(4 * DTL + 2 * N_M8, 64)
        .T
    )

    return [
        {
            "WM": WM_host,
            "W8": W8_host,
            "ABF": ABF_host[core],
            "A8": A8_host[core],
            "CT": CT_host[core],
            "BIAS": BIAS_host,
            "BIO": BIO_host,
        }
        for core in range(N_CORES)
    ]


def _postprocess(results):
    """results: per-core list of {'HT': [2048,1024], 'CNT': [2048,1024]}."""
    h_new = (
        np.stack([np.asarray(results[core]["HT"]) for core in range(N_CORES)])
        .transpose(0, 2, 1)
        .reshape(B, DH)
        .astype(np.float32)
    )
    c_new = (
        np.stack([np.asarray(results[core]["CNT"]) for core in range(N_CORES)])
        .transpose(0, 2, 1)
        .reshape(B, DH)
        .astype(np.float32)
    )
    return (h_new, c_new)


def kernel(x, h, c, Wix, bix, Wmx, bmx, Wox, box, Wih, bih, Wmh, bmh, Woh, boh):
    global LAST_RESULT
    from concourse.bass_utils import run_bass_kernel_spmd

    in_maps = _prepare_in_maps(
        x, h, c, Wix, bix, Wmx, bmx, Wox, box, Wih, bih, Wmh, bmh, Woh, boh
    )
    nc = _get_bass()
    try:
        res = run_bass_kernel_spmd(nc, in_maps, core_ids=list(range(N_CORES)))
    except ModuleNotFoundError:
        # BASS_TRACE under axon needs antenv.axon_hooks, which some
        # containers lack; fall back to an untraced run.
        os.environ["BASS_NEVER_TRACE"] = "1"
        res = run_bass_kernel_spmd(nc, in_maps, core_ids=list(range(N_CORES)))
    LAST_RESULT = res
    return _postprocess(res.results)



# revision 22
# speedup vs baseline: 1.0149x; 1.0149x over previous
"""LSTM cell kernel for Trainium2, SPMD over 8 NeuronCores.

Problem: nn_LstmCell — B=8192, D_IN=D_H=2048.
    g = x @ Wx.T + bx + h @ Wh.T + bh          # [B, 3H]
    gi, gm, go = split(g, 3)
    c_new = sigmoid(gm)*c + sigmoid(gi)*tanh(gm)
    h_new = sigmoid(go)*tanh(c_new)

Strategy:
  - Data-parallel over batch: each core owns 1024 rows of x/h/c.
  - Fused GEMM computed transposed (gates on PSUM partitions, batch on the
    free dim) so per-gate biases fold into the ScalarE activation.
  - Mixed precision split by gate sensitivity: the m-gate pre-activation
    feeds tanh (derivative ~1) so it runs in bf16; the i/o gates only feed
    sigmoid (derivative <= 0.25) so they run in fp8 e4m3 with DoubleRow
    perf mode (K=256 per matmul, 2x PE throughput). Measured end-to-end
    rel err ~1.4e-2 vs 2.6e-2 for all-fp8 (gate: 2e-2).
  - fp8 scaling: A*32, W*512 (W's absmax 0.022 is below e4m3's min normal
    0.0156, so unscaled W would quantize to subnormals). The 1/16384
    descale folds into the sigmoid activation's scale operand.
  - DoubleRow matmuls may only write PSUM partitions 0..63, so each i/o
    gate accumulates its two 64-row halves into separate [64, 512] banks;
    the sigmoid ACTs then write the two partition halves of one [128, 512]
    SBUF tile (ScalarE supports input/output partition-base offsets).
  - Weights streamed from HBM (one pass), activations resident in SBUF.

Host-side: layout transforms + bf16/fp8 casts (not counted in HW exec time).
"""

import os

import numpy as np
import ml_dtypes

N_CORES = 8
B = 8192
DH = 2048            # latent dim (= D_IN = D_H)
K = 2 * DH           # 4096 contraction dim
BLOC = B // N_CORES  # 1024 batch rows per core
P = 128
KT = K // P          # 32 k-tiles (128 each)
KG = KT // 2         # 16 doubled k-groups (256 each) for fp8 DoubleRow
DTL = DH // P        # 16 d-tiles per gate
NF = 512             # matmul free dim (one PSUM bank of fp32)
NH = BLOC // NF      # 2 batch halves (vtiles per d-tile)

N_M8 = 5             # leading m-gate d-tiles computed in fp8 (see below)
SA = 32.0            # fp8 activation scale
SW = 512.0           # fp8 weight scale
IO_DESCALE = 1.0 / (SA * SW)

_F8 = ml_dtypes.float8_e4m3

_CACHE = {}
LAST_RESULT = None  # BassKernelResults from the most recent run (for test.py)


def _split_multiwaits(nc):
    """This container's walrus build rejects >1 sync-wait on an engine
    instruction ("Too many sync wait commands"). Split extra waits into
    standalone EventSemaphore instructions on the same engine immediately
    before the instruction (same stall semantics: engines are in-order)."""
    import concourse.mybir as mybir

    f = nc.m.functions[0]
    for blk in f.blocks:
        new_insts = []
        for inst in blk.instructions:
            si = getattr(inst, "sync_info", None)
            ow = list(si.on_wait) if (si is not None and si.on_wait) else []
            if len(ow) > 1:
                for w in ow[:-1]:
                    new_insts.append(
                        mybir.InstEventSemaphore(
                            name=nc.get_next_instruction_name(),
                            engine=inst.engine,
                            ins=[],
                            outs=[],
                            sync_info=mybir.SyncInfo(on_wait=[w], on_update=[]),
                        )
                    )
                inst.sync_info = mybir.SyncInfo(
                    on_wait=[ow[-1]], on_update=list(si.on_update)
                )
            new_insts.append(inst)
        blk.instructions[:] = new_insts


def _build_bass(dtl=DTL):
    import concourse.bass as bass
    import concourse.mybir as mybir
    import concourse.tile as tile

    f32 = mybir.dt.float32
    f16 = mybir.dt.float16
    f8 = mybir.dt.float8e4
    AF = mybir.ActivationFunctionType
    DR = mybir.MatmulPerfMode.DoubleRow

    nc = bass.Bass("TRN2", name="lstm_cell")

    # m-gate weights, fp16: WM[d, p, kt, m] = Wm[d*128+m, kt*128+p]
    WM = nc.dram_tensor("WM", [dtl, P, KT, P], f16, kind="ExternalInput")
    # i/o gate weights, fp8 DoubleRow layout, plus N_M8 extra strips
    # (index 2*dtl+d) holding the M-GATE's first N_M8 d-tiles in fp8:
    # those tiles run all-fp8, halving their m-matmul time and decoupling
    # the prologue from the 8 MB bf16 A stream (which then loads during
    # the fp8 tiles instead of stalling the pipeline). Each converted tile
    # costs ~1e-3 of rel err (measured: 1 -> 1.51e-2, 3 -> 1.69e-2).
    # W8[g*16+d, p, kg, ii, m] = Wg[d*128+m, kg*256+ii*128+p] * SW
    W8 = nc.dram_tensor("W8", [2 * dtl + N_M8, P, KG, 2, P], f8, kind="ExternalInput")
    # activations A = [x ‖ h], twice: fp16 for m-gate, fp8*SA for i/o
    ABF = nc.dram_tensor("ABF", [P, KT, BLOC], f16, kind="ExternalInput")
    # A8 split vtile-major: d0 phase A (vtile 0) needs only half the stream
    A8 = nc.dram_tensor("A8", [NH, P, KT, NF], f8, kind="ExternalInput")
    CT = nc.dram_tensor("CT", [DH, BLOC], f32, kind="ExternalInput")
    BIAS = nc.dram_tensor("BIAS", [P, 3 * dtl], f32, kind="ExternalInput")
    # i/o biases regrouped per 64-row half (DoubleRow outputs sit at
    # partitions 0..63): BIO[p, ((g*16+d)*2)+b] = bias_g[d*128+b*64+p].
    # Cols 4*dtl + 2*d + b hold the m-gate's bias halves for the N_M8
    # fp8 m-tiles.
    BIO = nc.dram_tensor("BIO", [64, 4 * dtl + 2 * N_M8], f32, kind="ExternalInput")
    HT = nc.dram_tensor("HT", [DH, BLOC], f32, kind="ExternalOutput")
    CNT = nc.dram_tensor("CNT", [DH, BLOC], f32, kind="ExternalOutput")

    with tile.TileContext(nc) as tc:
        with (
            tc.tile_pool(name="const", bufs=1) as const_pool,
            tc.tile_pool(name="wpool", bufs=2) as wpool,
            tc.tile_pool(name="cpool", bufs=2) as cpool,
            tc.tile_pool(name="epool", bufs=3) as epool,
            tc.tile_pool(name="psum", bufs=1, space="PSUM") as psum_pool,
        ):
            # Activations resident in SBUF; per-k-chunk loads so the first
            # d-tile's matmuls can start as soon as early chunks land.
            # Both A streams go on the gpsimd queue, fp8 strictly first:
            # the model serializes all transfers through one shared DMA
            # resource, so only queue order keeps the big bf16 chunks (m-gate,
            # consumed last) from starving the fp8 chunks (i/o, consumed
            # first). Chunks sized to amortize the ~1us SWDGE prep per DMA.
            # vtile-major SBUF layout: per-partition runs of 4+ kts stay
            # contiguous (2 KB), keeping SWDGE descriptor counts low.
            a8_sb = const_pool.tile([P, NH, KT, NF], f8, name="a8_sb")
            # Few, growing chunks: each SWDGE prep costs ~1.1-1.7us of
            # Pool-ring time, so chunk count matters more than chunk size.
            A8_CH = [(0, 0, 2), (0, 2, 4), (0, 4, 8), (0, 8, 14), (0, 14, 20),
                     (0, 20, 32), (1, 0, 10), (1, 10, 20), (1, 20, 32)]
            for v, k0, k1 in A8_CH:
                nc.gpsimd.dma_start(a8_sb[:, v, k0:k1, :], A8[v][:, k0:k1, :])
            # ABF chunk DMAs are emitted inside the d==0 body, after d0's
            # wm/c loads, so those beat the bulk bf16 stream to the DMA
            # engines (the m-gate is scheduled last within d0 anyway).
            abf_sb = const_pool.tile([P, KT, BLOC], f16, name="abf_sb")
            # bias loads are emitted inside the d==0 body, after the first
            # weight strips: they aren't needed until the first epilogue,
            # and ahead of the strips they'd burn ~1.3us of HWDGE setup on
            # the critical path to the first matmul.
            bias_sb = const_pool.tile([P, 3 * dtl], f32, name="bias_sb")
            bio_sb = const_pool.tile([64, 4 * dtl + 2 * N_M8], f32, name="bio_sb")

            # ABF chunks drip onto the gpsimd ring behind the first four
            # epilogues' output DMAs (see epilogue()).
            abf_pending = [(4 * ch, 4 * ch + 4) for ch in range(8)]
            wm_prefetch = {}
            d1_pre = {}

            for d in range(dtl):
                # Stream this d-tile's weight strips: fp8 i/o (0.5 MB each)
                # + fp16 m (1 MB). Weight strips live on the sync queue,
                # which carries NO output DMAs (outputs ride the pool queue)
                # so strip preps never inherit end-of-tile semaphore waits.
                m_fp8 = d < min(N_M8, dtl)
                gates = [("i", d), ("o", dtl + d)]
                if m_fp8:
                    # i, m, o order matches d0's kg-major gate order
                    gates.insert(1 if d == 0 else 2, ("m", 2 * dtl + d))
                w8 = {}
                if d == 0:
                    # kg-pieces interleaved ACROSS gates so the kg-major
                    # phase-A loop can start after one short piece per gate.
                    for g, idx in gates:
                        w8[g] = wpool.tile(
                            [P, KG, 2, P], f8, name=f"w8{g}", tag=f"w8{g}"
                        )
                    for ip, (k0, k1) in enumerate(((0, 4), (4, 10), (10, 16))):
                        for g, idx in gates:
                            nc.sync.dma_start(
                                w8[g][:, k0:k1], W8[idx][:, k0:k1]
                            )
                        if ip == 0:
                            # bio gates epilogue(0) (which in turn gates all
                            # of phase B's psum-bank reuse); its tiny wire
                            # must rank early in the FIFO-by-ready-time
                            # arbiter or it starves behind the bulk streams.
                            # (bias_sb is only read from d5 on - loads later.)
                            nc.sync.dma_start(bio_sb[:], BIO[:])
                    # Prefetch d1's i-gate strip now: later sync emission
                    # would rank it behind the A8-v1 chunks in the
                    # FIFO-by-ready-time wire arbiter (needed ~15us in for
                    # d0 phase B's d1-i0 job).
                    w8i_d1 = wpool.tile([P, KG, 2, P], f8, name="w8i", tag="w8i")
                    for hf in range(2):
                        nc.sync.dma_start(
                            w8i_d1[:, 8 * hf : 8 * hf + 8],
                            W8[1][:, 8 * hf : 8 * hf + 8],
                        )
                    d1_pre["w8i"] = w8i_d1
                    nc.sync.dma_start(bias_sb[:], BIAS[:])
                else:
                    for g, idx in gates:
                        if d == 1 and g == "i":
                            w8[g] = d1_pre["w8i"]
                            continue
                        w8[g] = wpool.tile(
                            [P, KG, 2, P], f8, name=f"w8{g}", tag=f"w8{g}"
                        )
                        nc.sync.dma_start(w8[g][:], W8[idx])
                if d == 2:
                    # wm strips for d5/d6 have free wpool buffers, so their
                    # sync-queue DMAs would become wire-ready ~30us early and
                    # starve d2-d4's strips (FIFO-by-ready-time arbiter).
                    # Emit them on the gpsimd ring HERE, behind d1's output
                    # throttle, so they are ready only ~50us in.
                    for dd in (N_M8, N_M8 + 1):
                        t = wpool.tile([P, KT, P], f16, name="wm", tag="wm")
                        nc.gpsimd.dma_start(t[:], WM[dd])
                        wm_prefetch[dd] = t
                if not m_fp8:
                    wm = wm_prefetch.pop(d, None)
                    if wm is None:
                        wm = wpool.tile([P, KT, P], f16, name="wm", tag="wm")
                        nc.sync.dma_start(wm[:], WM[d])

                c_tiles, psums = {}, {}
                for nh in range(NH):
                    c_t = cpool.tile([P, NF], f32, name=f"c_{nh}", tag=f"c_{nh}")
                    nc.sync.dma_start(
                        c_t[:], CT[d * P : (d + 1) * P, nh * NF : (nh + 1) * NF]
                    )
                    c_tiles[nh] = c_t
                    # m-gate: one full bank; parity tags so consecutive
                    # vtiles overlap. i/o: [64, 512] banks (DoubleRow dst
                    # partition must be 0), one per 64-row half. d0's fp8
                    # m-gate gets its own [64, 512] pair (8 banks total).
                    if m_fp8:
                        for b in range(2):
                            psums[("m8", nh, b)] = psum_pool.tile(
                                [64, NF], f32, name=f"ps_m8{b}", tag=f"ps_m8{b}"
                            )
                    else:
                        psums[("m", nh)] = psum_pool.tile(
                            [P, NF], f32, name=f"ps_m{nh}", tag=f"ps_m{nh}"
                        )
                    for g in "io":
                        if d == 1 and g == "i" and nh == 0:
                            for b in range(2):
                                psums[(g, nh, b)] = d1_pre["i0psum"][b]
                            continue
                        if d == 0 and g == "i" and nh == 1:
                            continue  # allocated in the d0 branch, ps_i tags
                        for b in range(2):
                            psums[(g, nh, b)] = psum_pool.tile(
                                [64, NF], f32, name=f"ps_{g}{b}", tag=f"ps_{g}{b}"
                            )

                def io_matmul(g, nh, b, kg, key=None):
                    # fp8 DoubleRow: K=256 (k-tile pair), M=64, N=512.
                    nc.tensor.matmul(
                        psums[(key or g, nh, b)],
                        w8[g][:, kg, :, b * 64 : (b + 1) * 64],
                        a8_sb[:, nh, 2 * kg : 2 * kg + 2, :],
                        start=(kg == 0),
                        stop=(kg == KG - 1),
                        perf_mode=DR,
                    )

                def m_matmul(nh, kt):
                    nc.tensor.matmul(
                        psums[("m", nh)][:],
                        wm[:, kt, :],
                        abf_sb[:, kt, nh * NF : (nh + 1) * NF],
                        start=(kt == 0),
                        stop=(kt == KT - 1),
                    )

                def sig_io(g, gi, s_g, nh, b):
                    nc.scalar.activation(
                        s_g[b * 64 : (b + 1) * 64, :],
                        psums[(g, nh, b)],
                        AF.Sigmoid,
                        bias=bio_sb[
                            :, (gi * dtl + d) * 2 + b : (gi * dtl + d) * 2 + b + 1
                        ],
                        scale=IO_DESCALE,
                    )

                def epilogue(nh):
                    # Emission order matters: engines are in-order, so the
                    # o-dependent ops (s_o, h_new) go last — everything else
                    # completes during the o-gate matmuls and only the short
                    # s_o -> h_new chain trails the final matmul.
                    b_m = bias_sb[:, dtl + d : dtl + d + 1]

                    s_i = epool.tile([P, NF], f32, name="s_i", tag="s_i")
                    t_m = epool.tile([P, NF], f32, name="t_m", tag="t_m")
                    s_m = epool.tile([P, NF], f32, name="s_m", tag="s_m")
                    s_o = epool.tile([P, NF], f32, name="s_o", tag="s_o")
                    part = epool.tile([P, NF], f32, name="part", tag="part")
                    fc = epool.tile([P, NF], f32, name="fc", tag="fc")
                    c_new = epool.tile([P, NF], f32, name="c_new", tag="c_new")
                    t_c = epool.tile([P, NF], f32, name="t_c", tag="t_c")
                    h_new = epool.tile([P, NF], f32, name="h_new", tag="h_new")

                    # i halves: PSUM [64, 512] at partition base 0 ->
                    # partition halves of the [128, 512] SBUF tile.
                    for b in range(2):
                        sig_io("i", 0, s_i, nh, b)
                    if d == dtl - 1 and nh == NH - 1:
                        # Final vtile, o-gate LAST: its post-matmul chain is
                        # only s_o -> h -> DMA (3 hops), while the long
                        # m-chain (t_m/s_m -> part/fc -> c_new -> t_c) hides
                        # under the o-gate matmul window. m and o are both
                        # column-halved; each halving alternates psum banks
                        # so half 1's matmuls never wait on half 0's ACT
                        # reads of a shared tile.
                        NQ = NF // 2
                        mq_banks = [
                            psums[("m", nh)],
                            psum_pool.tile([P, NF], f32, name=f"ps_m{1 - nh}",
                                           tag=f"ps_m{1 - nh}"),
                        ]
                        for q in range(2):
                            cs = q * NQ
                            mq = mq_banks[q]
                            for kt in range(KT):
                                nc.tensor.matmul(
                                    mq[:, cs : cs + NQ],
                                    wm[:, kt, :],
                                    abf_sb[:, kt,
                                           nh * NF + cs : nh * NF + cs + NQ],
                                    start=(kt == 0),
                                    stop=(kt == KT - 1),
                                )
                            nc.scalar.activation(
                                t_m[:, cs : cs + NQ], mq[:, cs : cs + NQ],
                                AF.Tanh, bias=b_m)
                            nc.scalar.activation(
                                s_m[:, cs : cs + NQ], mq[:, cs : cs + NQ],
                                AF.Sigmoid, bias=b_m)
                            nc.vector.tensor_mul(
                                part[:, cs : cs + NQ], s_i[:, cs : cs + NQ],
                                t_m[:, cs : cs + NQ])
                            nc.vector.tensor_mul(
                                fc[:, cs : cs + NQ], s_m[:, cs : cs + NQ],
                                c_tiles[nh][:, cs : cs + NQ])
                            nc.vector.tensor_add(
                                c_new[:, cs : cs + NQ], fc[:, cs : cs + NQ],
                                part[:, cs : cs + NQ])
                            nc.scalar.activation(
                                t_c[:, cs : cs + NQ], c_new[:, cs : cs + NQ],
                                AF.Tanh)
                            nc.gpsimd.dma_start(
                                CNT[d * P : (d + 1) * P,
                                    nh * NF + cs : nh * NF + cs + NQ],
                                c_new[:, cs : cs + NQ])
                        m8pair = [psum_pool.tile([64, NF], f32,
                                                 name=f"ps_m8{b}", tag=f"ps_m8{b}")
                                  for b in range(2)]
                        opair = [psums[("o", nh, b)] for b in range(2)]
                        O_SL = [(0, 256, opair), (256, 128, m8pair),
                                (384, 128, opair)]
                        for q, (cs, w, obk) in enumerate(O_SL):
                            for kg in range(KG):
                                for b in range(2):
                                    nc.tensor.matmul(
                                        obk[b][:, cs : cs + w],
                                        w8["o"][:, kg, :, b * 64 : (b + 1) * 64],
                                        a8_sb[:, nh, 2 * kg : 2 * kg + 2,
                                              cs : cs + w],
                                        start=(kg == 0),
                                        stop=(kg == KG - 1),
                                        perf_mode=DR,
                                    )
                            for b in range(2):
                                col = (dtl + d) * 2 + b
                                nc.scalar.activation(
                                    s_o[b * 64 : (b + 1) * 64, cs : cs + w],
                                    obk[b][:, cs : cs + w],
                                    AF.Sigmoid,
                                    bias=bio_sb[:, col : col + 1],
                                    scale=IO_DESCALE,
                                )
                            nc.vector.tensor_mul(
                                h_new[:, cs : cs + w], s_o[:, cs : cs + w],
                                t_c[:, cs : cs + w])
                            heng = nc.gpsimd if q < 2 else nc.sync
                            heng.dma_start(
                                HT[d * P : (d + 1) * P,
                                   nh * NF + cs : nh * NF + cs + w],
                                h_new[:, cs : cs + w])
                        return
                    if m_fp8:
                        for b in range(2):
                            col = 4 * dtl + 2 * d + b
                            bm8 = bio_sb[:, col : col + 1]
                            nc.scalar.activation(
                                t_m[b * 64 : (b + 1) * 64, :],
                                psums[("m8", nh, b)][:],
                                AF.Tanh, bias=bm8, scale=IO_DESCALE,
                            )
                            nc.scalar.activation(
                                s_m[b * 64 : (b + 1) * 64, :],
                                psums[("m8", nh, b)][:],
                                AF.Sigmoid, bias=bm8, scale=IO_DESCALE,
                            )
                    else:
                        nc.scalar.activation(t_m[:], psums[("m", nh)][:], AF.Tanh, bias=b_m)
                        nc.scalar.activation(s_m[:], psums[("m", nh)][:], AF.Sigmoid, bias=b_m)
                    nc.vector.tensor_mul(part[:], s_i[:], t_m[:])
                    nc.vector.tensor_mul(fc[:], s_m[:], c_tiles[nh][:])
                    nc.vector.tensor_add(c_new[:], fc[:], part[:])
                    nc.scalar.activation(t_c[:], c_new[:], AF.Tanh)
                    # Outputs stay OFF the sync queue so later tiles' weight
                    # strip preps never queue behind output waits; the gpsimd
                    # SWDGE ring preps each output when its data lands, and
                    # doubles as a ready-time throttle for the ABF chunks
                    # emitted right after (wire arbitration is FIFO by ready
                    # time, so unthrottled bulk chunks would starve later
                    # weight strips).
                    nc.gpsimd.dma_start(
                        CNT[d * P : (d + 1) * P, nh * NF : (nh + 1) * NF], c_new[:]
                    )
                    for b in range(2):
                        sig_io("o", 1, s_o, nh, b)
                    nc.vector.tensor_mul(h_new[:], s_o[:], t_c[:])
                    nc.gpsimd.dma_start(
                        HT[d * P : (d + 1) * P, nh * NF : (nh + 1) * NF], h_new[:]
                    )
                    if abf_pending:
                        k0, k1 = abf_pending.pop(0)
                        nc.gpsimd.dma_start(
                            abf_sb[:, k0:k1, :], ABF[:, k0:k1, :]
                        )
                        k0, k1 = abf_pending.pop(0)
                        nc.gpsimd.dma_start(
                            abf_sb[:, k0:k1, :], ABF[:, k0:k1, :]
                        )

                if d == 0:
                    d1_i0 = []
                    for b in range(2):
                        t = psum_pool.tile([P, NF], f32, name=f"ps_m{b}", tag=f"ps_m{b}")
                        d1_i0.append(t[0:64, :])
                    d1_pre["i0psum"] = d1_i0

                    def d1i0_matmul(b, kg):
                        nc.tensor.matmul(
                            d1_i0[b],
                            d1_pre["w8i"][:, kg, :, b * 64 : (b + 1) * 64],
                            a8_sb[:, 0, 2 * kg : 2 * kg + 2, :],
                            start=(kg == 0),
                            stop=(kg == KG - 1),
                            perf_mode=DR,
                        )

                    # d0 is all-fp8 (m included), kg-major ACROSS gates so
                    # the PE tracks the A8 DMA stream without long stalls.
                    # Phase A covers vtile 0 only (halves the A8 bytes the
                    # prologue needs); phase B covers vtile 1, with an
                    # i-gate shim ahead so epilogue(0)'s ACTs can release
                    # the m/o PSUM banks while the PE stays busy. vtile 1's
                    # i-gate rides in the otherwise-idle pm banks ([64, 512]
                    # at base 0).
                    D1A = 11  # phase-A rounds that also run d1i0 (kg-D1A)
                    for kg in range(KG):
                        for g in "imo":
                            for b in range(2):
                                io_matmul(g, 0, b, kg, key="m8" if g == "m" else None)
                        if kg >= D1A:
                            for b in range(2):
                                d1i0_matmul(b, kg - D1A)
                    # Phase B is PE-bound with a 4th job: d1's i-gate vtile 0
                    # runs in the pm banks (its only deps: resident A8 v0 and
                    # the prefetched strip), absorbing A8-v1 wire burstiness.
                    # d0's i1 reuses the i-pair banks (freed by epilogue(0)'s
                    # s_i). The d1i0 shim covers epilogue(0)'s ACT latency
                    # before m1/o1 can reuse the m8/o banks.
                    for b in range(2):
                        psums[("i", 1, b)] = psum_pool.tile(
                            [64, NF], f32, name=f"ps_i{b}", tag=f"ps_i{b}"
                        )
                    SHIM = 9  # d1i0 kgs done by phase B round r: r + SHIM
                    epilogue(0)
                    for kg in range(KG - D1A, SHIM):
                        for b in range(2):
                            d1i0_matmul(b, kg)
                    for kg in range(KG):
                        for b in range(2):
                            io_matmul("i", 1, b, kg)
                        for g in "mo":
                            for b in range(2):
                                io_matmul(g, 1, b, kg, key="m8" if g == "m" else None)
                        if kg + SHIM < KG:
                            for b in range(2):
                                d1i0_matmul(b, kg + SHIM)
                    epilogue(1)
                else:
                    # gate-major per vtile, o last: everything except the
                    # short s_o -> h_new chain completes during the o-gate
                    # matmuls (see epilogue()). The very last vtile's o
                    # matmuls are emitted column-halved inside epilogue().
                    for nh in range(NH):
                        if not (d == 1 and nh == 0):
                            for b in range(2):
                                for kg in range(KG):
                                    io_matmul("i", nh, b, kg)
                        if m_fp8:
                            for b in range(2):
                                for kg in range(KG):
                                    io_matmul("m", nh, b, kg, key="m8")
                        elif not (d == dtl - 1 and nh == NH - 1):
                            # final vtile's m matmuls are emitted inside
                            # epilogue(), column-quartered with the chain
                            for kt in range(KT):
                                m_matmul(nh, kt)
                        if not (d == dtl - 1 and nh == NH - 1):
                            for b in range(2):
                                for kg in range(KG):
                                    io_matmul("o", nh, b, kg)
                        epilogue(nh)

    _split_multiwaits(nc)
    return nc


def _get_bass():
    if "nc" not in _CACHE:
        _CACHE["nc"] = _build_bass()
    return _CACHE["nc"]


def _prepare_in_maps(x, h, c, Wix, bix, Wmx, bmx, Wox, box, Wih, bih, Wmh, bmh, Woh, boh):
    x = np.asarray(x, dtype=np.float32)
    h = np.asarray(h, dtype=np.float32)
    c = np.asarray(c, dtype=np.float32)

    # Per-gate fused weights [2048, 4096]: W = [Wx ‖ Wh]
    Wg = {
        "i": np.concatenate([np.asarray(Wix), np.asarray(Wih)], axis=1),
        "m": np.concatenate([np.asarray(Wmx), np.asarray(Wmh)], axis=1),
        "o": np.concatenate([np.asarray(Wox), np.asarray(Woh)], axis=1),
    }

    # m-gate bf16: WM[d, p, kt, m] = Wm[d*128+m, kt*128+p]
    WM_host = np.ascontiguousarray(
        Wg["m"].astype(np.float32).reshape(DTL, P, KT, P).transpose(0, 3, 2, 1)
    ).astype(np.float16)

    # i/o gates fp8 (scaled by SW), DoubleRow layout:
    # W8[g*16+d, p, kg, ii, m] = Wg[d*128+m, kg*256+ii*128+p]*SW
    # plus the m-gate's d=0 strip at index 2*DTL (d-tile 0 runs all-fp8).
    w8_list = []
    for g in "io":
        ws = (Wg[g].astype(np.float32) * SW).astype(_F8)
        w8_list.append(ws.reshape(DTL, P, KG, 2, P).transpose(0, 4, 2, 3, 1))
    wm8 = (Wg["m"][: N_M8 * P].astype(np.float32) * SW).astype(_F8)
    w8_list.append(wm8.reshape(N_M8, P, KG, 2, P).transpose(0, 4, 2, 3, 1))
    W8_host = np.ascontiguousarray(np.concatenate(w8_list, axis=0))

    # A = [x ‖ h] : [8192, 4096] -> per-core [p, kt, n] fp16, and
    # vtile-major [v, p, kt, n_half] fp8*SA
    A = np.concatenate([x, h], axis=1)
    A_t = A.reshape(N_CORES, BLOC, KT, P).transpose(0, 3, 2, 1)
    ABF_host = np.ascontiguousarray(A_t).astype(np.float16)
    A8_host = np.ascontiguousarray(
        (A_t * np.float32(SA))
        .reshape(N_CORES, P, KT, NH, NF)
        .transpose(0, 3, 1, 2, 4)
    ).astype(_F8)

    # c transposed per core: [core, 2048, 1024]
    CT_host = np.ascontiguousarray(c.reshape(N_CORES, BLOC, DH).transpose(0, 2, 1))

    bias = {g: (np.asarray(bx) + np.asarray(bh)).astype(np.float32)
            for g, bx, bh in (("i", bix, bih), ("m", bmx, bmh), ("o", box, boh))}
    BIAS_host = np.ascontiguousarray(
        np.concatenate([bias["i"], bias["m"], bias["o"]]).reshape(3 * DTL, P).T
    )
    # BIO[p, (g*16+d)*2+b] = bias_g[d*128+b*64+p] for g in (i, o);
    # trailing 2*N_M8 cols: m-gate bias halves for the fp8 m-tiles.
    BIO_host = np.ascontiguousarray(
        np.concatenate([bias["i"], bias["o"], bias["m"][: N_M8 * P]])
        .reshape(4 * DTL + 2 * N_M8, 64)
        .T
    )

    return [
        {
            "WM": WM_host,
            "W8": W8_host,
            "ABF": ABF_host[core],
            "A8": A8_host[core],
            "CT": CT_host[core],
            "BIAS": BIAS_host,
            "BIO": BIO_host,
        }
        for core in range(N_CORES)
    ]


def _postprocess(results):
    """results: per-core list of {'HT': [2048,1024], 'CNT': [2048,1024]}."""
    h_new = (
        np.stack([np.asarray(results[core]["HT"]) for core in range(N_CORES)])
        .transpose(0, 2, 1)
        .reshape(B, DH)
        .astype(np.float32)
    )
    c_new = (
        np.stack([np.asarray(results[core]["CNT"]) for core in range(N_CORES)])
        .transpose(0, 2, 1)
        .reshape(B, DH)
        .astype(np.float32)
    )
    return (h_new, c_new)


def kernel(x, h, c, Wix, bix, Wmx, bmx, Wox, box, Wih, bih, Wmh, bmh, Woh, boh):
    global LAST_RESULT
    from concourse.bass_utils import run_bass_kernel_spmd

    in_maps = _prepare_in_maps(
        x, h, c, Wix, bix, Wmx, bmx, Wox, box, Wih, bih, Wmh, bmh, Woh, boh
    )
    nc = _get_bass()
    try:
        res = run_bass_kernel_spmd(nc, in_maps, core_ids=list(range(N_CORES)))
    except ModuleNotFoundError:
        # BASS_TRACE under axon needs antenv.axon_hooks, which some
        # containers lack; fall back to an untraced run.
        os.environ["BASS_NEVER_TRACE"] = "1"
        res = run_bass_kernel_spmd(nc, in_maps, core_ids=list(range(N_CORES)))
    LAST_RESULT = res
    return _postprocess(res.results)

